# revision 7
# baseline (speedup 1.0000x reference)
"""Trainium2 Bass kernel for AlignmentContrastiveLoss (8-core SPMD, label-sharded).

Math: with conserved c_i = (cat_i < 3), key k_i = (label_i - 64*core)*16 +
graph_i (non-conserved rows dropped),

  pos_cnt        = 1/2 (sum_L n_L^2 - sum_k n_k^2)
  sum_valid_sims = 1/2 (||U||_F^2 - ||W||_F^2)
      U[L,:] = sum_{i: l_i=L, c_i} e_i   (e = row-normalized embeddings)
      W[k,:] = sum_{i: k_i=k, c_i} e_i
  pos_sum        = pos_cnt - sum_valid_sims

Sharding: conserved rows are bucketed BY LABEL on the host -- core c owns
labels [64c, 64c+64).  Positive pairs require equal labels, so every term
above is core-local: ||U||^2, ||W||^2, sum n_L^2 and sum n_k^2 all reduce
to per-core scalars, and the only collective is a 2 KB AllReduce of six
f32 scalars.  Per core the 1024 keys split into 8 blocks of 128; rows are
host-packed 128 slots per block (occupancy ~66; overflow needs +8 sigma),
so W is 8 single-tile one-hot matmuls with 1/||r|| folded into the
one-hot.  U = sel^T W via 8 more matmuls against a constant label-
selection matrix.  Negative pairs are sharded 625/core with raw rows
host-gathered (pure indexing); sims use fused multiply-reduce.
"""

import os
import sys

import numpy as np

if "/opt/trn_rl_repo" not in sys.path:
    sys.path.insert(0, "/opt/trn_rl_repo")

# persistent jax/neuron compile cache: repeat invocations skip the NEFF build
os.environ.setdefault("JAX_COMPILATION_CACHE_DIR", "/tmp/jaxcache")
os.environ.setdefault("JAX_PERSISTENT_CACHE_MIN_COMPILE_TIME_SECS", "1")
os.environ.setdefault("JAX_PERSISTENT_CACHE_MIN_ENTRY_SIZE_BYTES", "0")

import concourse.mybir as mybir  # noqa: E402
import concourse.tile as tile  # noqa: E402
from concourse import bacc  # noqa: E402
from concourse.bass_utils import run_bass_kernel_spmd  # noqa: E402

# Problem constants (hardcoded per the self-contained-kernel contract).
N, D, S = 8192, 512, 5000
M = 8                 # cores
NB = 8                # key blocks per core (128 keys each; 1024 keys/core)
OWN = NB * 128        # own-row slots per core (128 per key block)
SP = S // M           # 625 pairs per core
NPT = 5               # neg pair tiles: 5 * 128 = 640 >= 625
LPC = 64              # labels per core

F32 = mybir.dt.float32
BF16 = mybir.dt.bfloat16
I16 = mybir.dt.int16
ALU = mybir.AluOpType
ACTF = mybir.ActivationFunctionType
AX = mybir.AxisListType

_PROGRAM_CACHE = {}


def build_program():
    """Build + compile the (single) SPMD Bass program. Returns nc."""
    if "nc" in _PROGRAM_CACHE:
        return _PROGRAM_CACHE["nc"]

    nc = bacc.Bacc("TRN2", target_bir_lowering=False, debug=False, num_devices=M)

    own_d = nc.dram_tensor("own", [OWN, D], BF16, kind="ExternalInput")
    nr1_d = nc.dram_tensor("nr1", [128, NPT, D], BF16, kind="ExternalInput")
    nr2_d = nc.dram_tensor("nr2", [128, NPT, D], BF16, kind="ExternalInput")
    mf_d = nc.dram_tensor("mf", [128, 48], F32, kind="ExternalInput")
    selb_d = nc.dram_tensor("selb", [128, 8 + NB * LPC], BF16, kind="ExternalInput")
    out_d = nc.dram_tensor("out", [1, 1], F32, kind="ExternalOutput")

    groups = [list(range(M))]

    with tile.TileContext(nc) as tc:
        with (
            tc.tile_pool(name="cst", bufs=1) as cst,
            tc.tile_pool(name="sb", bufs=2) as sb,
            tc.tile_pool(name="drp", bufs=1, space="DRAM") as drp,
        ):
            # ---- constants / inputs (7 DMAs total in the whole kernel) ----
            iota_t = cst.tile([128, 128], I16, name="iota_t")
            nc.gpsimd.iota(iota_t[:], pattern=[[1, 128]], base=0, channel_multiplier=0)
            ones_bf = cst.tile([128, 1], BF16, name="ones_bf")
            nc.vector.memset(ones_bf[:], 1.0)
            ones_f32 = cst.tile([128, 1], F32, name="ones_f32")
            nc.vector.memset(ones_f32[:], 1.0)
            epsb = cst.tile([128, 1], F32, name="epsb")
            nc.vector.memset(epsb[:], 1e-12)

            mf = cst.tile([128, 48], F32, name="mf")   # krel | neg meta | lab
            nc.sync.dma_start(mf[:], mf_d[:, :])
            selb = cst.tile([128, 8 + NB * LPC], BF16, name="selb")
            nc.gpsimd.dma_start(selb[:], selb_d[:, :])
            # ---- phase A: row sumsq + one-hots + inv-norms ----
            # own-row DMAs issued FIRST: they gate the scalar-norm chain,
            # while the (larger) neg-pair rows are not needed until later
            o_sum = cst.tile([128, NB * 128], BF16, name="o_sum")
            sqall = cst.tile([128, NB], F32, name="sqall")
            e_ts = []
            dma_engs = [nc.sync, nc.gpsimd]
            for j in range(NB):
                e_t = cst.tile([128, D], BF16, name=f"e_{j}")
                dma_engs[j % 2].dma_start(e_t[:], own_d[j * 128 : (j + 1) * 128, :])
                e_ts.append(e_t)
            # g1 as 5 flat tiles (feeds scalar-side sumsq); g2 stays 3D
            g1t = []
            for t in range(NPT):
                g = cst.tile([128, D], BF16, name=f"g1_{t}")
                nc.scalar.dma_start(g[:], nr1_d[:, t, :])
                g1t.append(g)
            g2 = cst.tile([128, NPT, D], BF16, name="g2")
            nc.gpsimd.dma_start(g2[:], nr2_d[:, :, :])
            for j in range(NB):
                scr = sb.tile([128, D], F32, name=f"scr_{j}", tag="scr", bufs=4)
                nc.scalar.activation(
                    scr[:], e_ts[j][:], ACTF.Square, accum_out=sqall[:, j : j + 1]
                )
                nc.vector.tensor_scalar(
                    o_sum[:, j * 128 : (j + 1) * 128],
                    iota_t[:],
                    mf[:, j : j + 1],
                    None,
                    ALU.is_equal,
                )
            # inv = 1/sqrt(max(ss, eps)); pad rows (ss=0) stay finite, one-hot=0
            sqc = cst.tile([128, NB], F32, name="sqc")
            nc.vector.tensor_scalar(sqc[:], sqall[:], 1e-12, None, ALU.max)
            nrm = cst.tile([128, NB], F32, name="nrm")
            nc.scalar.activation(nrm[:], sqc[:], ACTF.Sqrt)
            inv = cst.tile([128, NB], F32, name="inv")
            nc.vector.reciprocal(inv[:], nrm[:])

            # ---- phase B: W one-hot matmuls (scaled one-hot x raw rows) ----
            pspW_cm = tc.tile_pool(name="pspW", bufs=1, space="PSUM")
            pspW = pspW_cm.__enter__()
            pw_all = pspW.tile([128, NB * D], F32, name="pw_all")
            for j in range(NB):
                soh = sb.tile([128, 128], BF16, name=f"soh_{j}", tag="soh", bufs=4)
                nc.vector.tensor_scalar(
                    soh[:], o_sum[:, j * 128 : (j + 1) * 128], inv[:, j : j + 1],
                    None, ALU.mult,
                )
                nc.tensor.matmul(
                    pw_all[:, j * D : (j + 1) * D], soh[:], e_ts[j][:],
                    start=True, stop=True,
                )

            # ---- phase C: evacuate W (bf16 copies) + one big ||W||^2 ----
            wsqc = sb.tile([128, 1], F32, name="wsqc")
            wscr = sb.tile([128, NB * D], F32, name="wscr")
            nc.scalar.activation(
                wscr[:], pw_all[:, :], ACTF.Square, accum_out=wsqc[:]
            )
            w_sb = []
            for b in range(NB):
                w_t = sb.tile([128, D], BF16, name=f"w_{b}", tag=f"wsb{b}")
                if b % 2 == 0:
                    nc.vector.tensor_copy(w_t[:], pw_all[:, b * D : (b + 1) * D])
                else:
                    nc.scalar.activation(
                        w_t[:], pw_all[:, b * D : (b + 1) * D], ACTF.Copy
                    )
                w_sb.append(w_t)
            pspW_cm.__exit__(None, None, None)

            # ---- phase D: counts + U from evacuated W ----
            psp2_cm = tc.tile_pool(name="psp2", bufs=1, space="PSUM")
            psp2 = psp2_cm.__enter__()
            # partials gathered as columns of redcol, one ones-matmul at the end
            # cols: 0=||W||^2 1=neg_sum 2=neg_cnt 3=sum nk^2 4=||U||^2 5=sum nL^2
            redcol = sb.tile([128, 8], F32, name="redcol")
            nc.vector.memset(redcol[:], 0.0)

            # n_k: column sums of the one-hot -> [1, 1024] (two bank-aligned
            # matmuls, as in the proven baseline construct)
            psc = psp2.tile([1, 1024], F32, name="psc")
            nc.tensor.matmul(
                psc[0:1, 0:512], ones_bf[:], o_sum[:, 0:512], start=True, stop=True
            )
            nc.tensor.matmul(
                psc[0:1, 512:1024], ones_bf[:], o_sum[:, 512:1024],
                start=True, stop=True,
            )
            cscr = sb.tile([1, 1024], F32, name="cscr")
            nk2t = sb.tile([1, 1], F32, name="nk2t")
            nc.scalar.activation(cscr[:], psc[0:1, :], ACTF.Square, accum_out=nk2t[:])
            # n_L via label one-hots (mf cols 40:48 hold each slot's label 0-63)
            psnl = psp2.tile([1, LPC], F32, name="psnl")
            for b in range(NB):
                olab = sb.tile([128, LPC], BF16, name=f"olab_{b}", tag="olab", bufs=2)
                nc.vector.tensor_scalar(
                    olab[:], iota_t[:, 0:LPC], mf[:, 40 + b : 41 + b], None,
                    ALU.is_equal,
                )
                nc.tensor.matmul(
                    psnl[:, :], ones_bf[:], olab[:],
                    start=(b == 0), stop=(b == NB - 1),
                )
            lscr = sb.tile([1, LPC], F32, name="lscr")
            nl2t = sb.tile([1, 1], F32, name="nl2t")
            nc.scalar.activation(lscr[:], psnl[0:1, :], ACTF.Square, accum_out=nl2t[:])
            # U[64, 512] accumulated over blocks via label-selection matrix
            psU = psp2.tile([LPC, D], F32, name="psU")
            for b in range(NB):
                nc.tensor.matmul(
                    psU[:, :],
                    selb[:, 8 + b * LPC : 8 + (b + 1) * LPC],
                    w_sb[b][:],
                    start=(b == 0),
                    stop=(b == NB - 1),
                )
            uscr = sb.tile([LPC, D], F32, name="uscr")
            u2t = sb.tile([LPC, 1], F32, name="u2t")
            nc.scalar.activation(uscr[:], psU[:, :], ACTF.Square, accum_out=u2t[:])

            # ---- phase E: negative pairs (proven mult + reduce on 3D tiles) ----
            ss1 = sb.tile([128, NPT], F32, name="ss1")
            ss2 = sb.tile([128, NPT], F32, name="ss2")
            dots = sb.tile([128, NPT], F32, name="dots")
            for t in range(NPT):
                nsc = sb.tile([128, D], F32, name=f"nsc_{t}", tag="nsc", bufs=2)
                nc.scalar.activation(
                    nsc[:], g1t[t][:], ACTF.Square, accum_out=ss1[:, t : t + 1]
                )
            sq2 = sb.tile([128, NPT, D], BF16, name="sq2")
            nc.vector.tensor_tensor(sq2[:], g2[:], g2[:], ALU.mult)
            nc.vector.tensor_reduce(ss2[:], sq2[:], axis=AX.X, op=ALU.add)
            prod = sb.tile([128, NPT, D], BF16, name="prod")
            for t in range(NPT):
                nc.vector.tensor_tensor(
                    prod[:, t, :], g1t[t][:], g2[:, t, :], ALU.mult
                )
            nc.vector.tensor_reduce(dots[:], prod[:], axis=AX.X, op=ALU.add)

            ssp = sb.tile([128, NPT], F32, name="ssp")
            nc.vector.tensor_tensor(ssp[:], ss1[:], ss2[:], ALU.mult)
            nc.vector.tensor_scalar(ssp[:], ssp[:], 1e-12, None, ALU.max)
            s12 = sb.tile([128, NPT], F32, name="s12")
            nc.scalar.activation(s12[:], ssp[:], ACTF.Sqrt)
            inv12 = sb.tile([128, NPT], F32, name="inv12")
            nc.vector.reciprocal(inv12[:], s12[:])
            sim = sb.tile([128, NPT], F32, name="sim")
            nc.vector.tensor_tensor(sim[:], dots[:], inv12[:], ALU.mult)
            pen = sb.tile([128, NPT], F32, name="pen")
            nc.vector.tensor_scalar(pen[:], sim[:], 0.0, None, ALU.max)

            # masks: (l1 != l2) & (g1 != g2) & (cons1 | cons2); meta at mf[:,8:]
            mo = 8
            vmask = sb.tile([128, NPT], F32, name="vmask")
            nc.vector.tensor_tensor(
                vmask[:], mf[:, mo : mo + NPT], mf[:, mo + NPT : mo + 2 * NPT],
                ALU.not_equal,
            )
            gmask = sb.tile([128, NPT], F32, name="gmask")
            nc.vector.tensor_tensor(
                gmask[:], mf[:, mo + 2 * NPT : mo + 3 * NPT],
                mf[:, mo + 3 * NPT : mo + 4 * NPT], ALU.not_equal,
            )
            nc.vector.tensor_tensor(vmask[:], vmask[:], gmask[:], ALU.mult)
            c1c = sb.tile([128, NPT], F32, name="c1c")
            nc.vector.tensor_scalar(
                c1c[:], mf[:, mo + 4 * NPT : mo + 5 * NPT], 2.5, None, ALU.is_lt
            )
            c2c = sb.tile([128, NPT], F32, name="c2c")
            nc.vector.tensor_scalar(
                c2c[:], mf[:, mo + 5 * NPT : mo + 6 * NPT], 2.5, None, ALU.is_lt
            )
            nc.vector.tensor_tensor(c1c[:], c1c[:], c2c[:], ALU.add)
            cmask = sb.tile([128, NPT], F32, name="cmask")
            nc.vector.tensor_scalar(cmask[:], c1c[:], 0.5, None, ALU.is_gt)
            nc.vector.tensor_tensor(vmask[:], vmask[:], cmask[:], ALU.mult)
            nc.vector.tensor_tensor(pen[:], pen[:], vmask[:], ALU.mult)

            # ---- phase F: gather partials into one row via a ones-matmul ----
            nc.vector.tensor_copy(redcol[:, 0:1], wsqc[:])
            nc.vector.tensor_reduce(redcol[:, 1:2], pen[:], axis=AX.X, op=ALU.add)
            nc.vector.tensor_reduce(redcol[:, 2:3], vmask[:], axis=AX.X, op=ALU.add)
            nc.vector.tensor_copy(redcol[0:1, 3:4], nk2t[:])
            nc.vector.tensor_copy(redcol[0:LPC, 4:5], u2t[:])
            nc.vector.tensor_copy(redcol[0:1, 5:6], nl2t[:])
            psum_s = psp2.tile([1, 8], F32, name="psum_s")
            nc.tensor.matmul(
                psum_s[0:1, 0:8], ones_f32[:], redcol[:], start=True, stop=True
            )
            # ---- phase G: 2 KB all-reduce of the six scalars ----
            arb = drp.tile([8, 512], F32, name="arb")
            arbz = sb.tile([8, 512], F32, name="arbz")
            nc.vector.memset(arbz[:], 0.0)
            nc.vector.tensor_copy(arbz[0:1, 0:8], psum_s[0:1, 0:8])
            nc.sync.dma_start(arb[:, :], arbz[:])
            aro = drp.tile([8, 512], F32, name="aro", addr_space="Shared")
            nc.gpsimd.collective_compute(
                "AllReduce",
                ALU.add,
                replica_groups=groups,
                ins=[arb.opt()],
                outs=[aro.opt()],
            )

            # ---- phase H: final scalar ----
            scf = sb.tile([1, 512], F32, name="scf")
            nc.sync.dma_start(scf[:], aro[0:1, :])
            # pos_cnt = 0.5*(sum nL^2 - sum nk^2)
            pc = sb.tile([1, 1], F32, name="pc")
            nc.vector.tensor_tensor(pc[:], scf[:, 5:6], scf[:, 3:4], ALU.subtract)
            nc.vector.tensor_scalar(pc[:], pc[:], 0.5, None, ALU.mult)
            # pos_sumsim = 0.5*(||U||^2 - ||W||^2)
            ps_ = sb.tile([1, 1], F32, name="ps_")
            nc.vector.tensor_tensor(ps_[:], scf[:, 4:5], scf[:, 0:1], ALU.subtract)
            nc.vector.tensor_scalar(ps_[:], ps_[:], 0.5, None, ALU.mult)
            # pos_loss = (pos_cnt - pos_sumsim) / max(pos_cnt,1) * (pos_cnt>0)
            psum_t = sb.tile([1, 1], F32, name="psum_t")
            nc.vector.tensor_tensor(psum_t[:], pc[:], ps_[:], ALU.subtract)
            den = sb.tile([1, 1], F32, name="den")
            nc.vector.tensor_scalar(den[:], pc[:], 1.0, None, ALU.max)
            rec = sb.tile([1, 1], F32, name="rec")
            nc.vector.reciprocal(rec[:], den[:])
            msk = sb.tile([1, 1], F32, name="msk")
            nc.vector.tensor_scalar(msk[:], pc[:], 0.0, None, ALU.is_gt)
            ploss = sb.tile([1, 1], F32, name="ploss")
            nc.vector.scalar_tensor_tensor(
                ploss[:], psum_t[:], rec[:], msk[:], ALU.mult, ALU.mult
            )
            # neg_loss
            den2 = sb.tile([1, 1], F32, name="den2")
            nc.vector.tensor_scalar(den2[:], scf[:, 2:3], 1.0, None, ALU.max)
            rec2 = sb.tile([1, 1], F32, name="rec2")
            nc.vector.reciprocal(rec2[:], den2[:])
            msk2 = sb.tile([1, 1], F32, name="msk2")
            nc.vector.tensor_scalar(msk2[:], scf[:, 2:3], 0.0, None, ALU.is_gt)
            nloss = sb.tile([1, 1], F32, name="nloss")
            nc.vector.scalar_tensor_tensor(
                nloss[:], scf[:, 1:2], rec2[:], msk2[:], ALU.mult, ALU.mult
            )

            outv = sb.tile([1, 1], F32, name="outv")
            nc.vector.tensor_tensor(outv[:], ploss[:], nloss[:], ALU.add)
            nc.sync.dma_start(out_d[:, :], outv[:])
            psp2_cm.__exit__(None, None, None)

    nc.compile()
    _PROGRAM_CACHE["nc"] = nc
    return nc


def make_in_maps(embeddings, labels, graph_ids, categories, idx1, idx2):
    """Host-side sharding / layout marshaling. Returns per-core input dicts."""
    import ml_dtypes

    emb = np.ascontiguousarray(
        np.asarray(embeddings, dtype=np.float32).astype(ml_dtypes.bfloat16)
    )
    l = np.asarray(labels).astype(np.int64)
    g = np.asarray(graph_ids).astype(np.int64)
    c = np.asarray(categories).astype(np.int64)
    i1 = np.asarray(idx1).astype(np.int64)
    i2 = np.asarray(idx2).astype(np.int64)
    assert emb.shape == (N, D) and l.shape == (N,) and i1.shape == (S,)

    cons = c < 3
    p_ar = np.arange(128)
    selb = np.zeros((128, 8 + NB * LPC), dtype=ml_dtypes.bfloat16)
    selb[:, 0:8] = p_ar[:, None] // 16 == np.arange(8)[None, :]
    for b in range(NB):
        selb[:, 8 + b * LPC : 8 + (b + 1) * LPC] = (
            (8 * b + p_ar[:, None] // 16) == np.arange(LPC)[None, :]
        )

    in_maps = []
    for core in range(M):
        own = np.zeros((NB, 128, D), dtype=ml_dtypes.bfloat16)
        mf = np.zeros((128, 48), dtype=np.float32)
        mf[:, 0:8] = 999.0
        mf[:, 40:48] = 999.0
        for b in range(NB):
            lo = 64 * core + 8 * b
            sel = np.nonzero(cons & (l >= lo) & (l < lo + 8))[0]
            nb_ = len(sel)
            assert nb_ <= 128, f"key-block overflow: {nb_} rows"
            own[b, :nb_] = emb[sel]
            mf[:nb_, b] = ((l[sel] - lo) * 16 + g[sel]).astype(np.float32)
            mf[:nb_, 40 + b] = (l[sel] - 64 * core).astype(np.float32)

        # negative pairs: q-th pair of this core at [q % 128, q // 128]
        sl = slice(core * SP, (core + 1) * SP)
        p1 = np.zeros(NPT * 128, np.int64)
        p2 = np.zeros(NPT * 128, np.int64)
        p1[:SP] = i1[sl]
        p2[:SP] = i2[sl]
        nr1 = np.ascontiguousarray(emb[p1].reshape(NPT, 128, D).transpose(1, 0, 2))
        nr2 = np.ascontiguousarray(emb[p2].reshape(NPT, 128, D).transpose(1, 0, 2))
        for f, arr in enumerate((l[p1], l[p2], g[p1], g[p2], c[p1], c[p2])):
            mf[:, 8 + f * NPT : 8 + (f + 1) * NPT] = arr.reshape(NPT, 128).T
        # pad pairs (q >= SP): force-invalid via equal labels
        padmask = np.zeros(NPT * 128, bool)
        padmask[SP:] = True
        pm2 = padmask.reshape(NPT, 128).T
        mf[:, 8 : 8 + NPT][pm2] = 0.0
        mf[:, 8 + NPT : 8 + 2 * NPT][pm2] = 0.0

        in_maps.append(
            {
                "own": own.reshape(OWN, D),
                "nr1": nr1,
                "nr2": nr2,
                "mf": mf,
                "selb": selb,
            }
        )
    return in_maps


def kernel(embeddings, labels, graph_ids, categories, idx1, idx2):
    nc = build_program()
    in_maps = make_in_maps(embeddings, labels, graph_ids, categories, idx1, idx2)
    res = run_bass_kernel_spmd(nc, in_maps, list(range(M)))
    out = np.asarray(res.results[0]["out"], dtype=np.float32)
    return out.reshape(())


# revision 8
# speedup vs baseline: 1.0367x; 1.0367x over previous
"""Trainium2 Bass kernel for AlignmentContrastiveLoss (8-core SPMD, label-sharded).

Math: with conserved c_i = (cat_i < 3), key k_i = (label_i - 64*core)*16 +
graph_i (non-conserved rows dropped),

  pos_cnt        = 1/2 (sum_L n_L^2 - sum_k n_k^2)
  sum_valid_sims = 1/2 (||U||_F^2 - ||W||_F^2)
      U[L,:] = sum_{i: l_i=L, c_i} e_i   (e = row-normalized embeddings)
      W[k,:] = sum_{i: k_i=k, c_i} e_i
  pos_sum        = pos_cnt - sum_valid_sims

Sharding: conserved rows are bucketed BY LABEL on the host -- core c owns
labels [64c, 64c+64).  Positive pairs require equal labels, so every term
above is core-local: ||U||^2, ||W||^2, sum n_L^2 and sum n_k^2 all reduce
to per-core scalars, and the only collective is a 2 KB AllReduce of six
f32 scalars.  Per core the 1024 keys split into 8 blocks of 128; rows are
host-packed 128 slots per block (occupancy ~66; overflow needs +8 sigma),
so W is 8 single-tile one-hot matmuls with 1/||r|| folded into the
one-hot.  U = sel^T W via 8 more matmuls against a constant label-
selection matrix.  Negative pairs are sharded 625/core with raw rows
host-gathered (pure indexing); sims use fused multiply-reduce.
"""

import os
import sys

import numpy as np

if "/opt/trn_rl_repo" not in sys.path:
    sys.path.insert(0, "/opt/trn_rl_repo")

# persistent jax/neuron compile cache: repeat invocations skip the NEFF build
os.environ.setdefault("JAX_COMPILATION_CACHE_DIR", "/tmp/jaxcache")
os.environ.setdefault("JAX_PERSISTENT_CACHE_MIN_COMPILE_TIME_SECS", "1")
os.environ.setdefault("JAX_PERSISTENT_CACHE_MIN_ENTRY_SIZE_BYTES", "0")

import concourse.mybir as mybir  # noqa: E402
import concourse.tile as tile  # noqa: E402
from concourse import bacc  # noqa: E402
from concourse.bass_utils import run_bass_kernel_spmd  # noqa: E402

# Problem constants (hardcoded per the self-contained-kernel contract).
N, D, S = 8192, 512, 5000
M = 8                 # cores
NB = 8                # key blocks per core (128 keys each; 1024 keys/core)
OWN = NB * 96         # own-row slots per core (96 per key block)
OSL = 96              # slots per key block (max observed occupancy 89)
SP = S // M           # 625 pairs per core
NPT = 5               # neg pair tiles: 5 * 128 = 640 >= 625
LPC = 64              # labels per core

F32 = mybir.dt.float32
BF16 = mybir.dt.bfloat16
I16 = mybir.dt.int16
ALU = mybir.AluOpType
ACTF = mybir.ActivationFunctionType
AX = mybir.AxisListType

_PROGRAM_CACHE = {}


def build_program():
    """Build + compile the (single) SPMD Bass program. Returns nc."""
    if "nc" in _PROGRAM_CACHE:
        return _PROGRAM_CACHE["nc"]

    nc = bacc.Bacc("TRN2", target_bir_lowering=False, debug=False, num_devices=M)

    own_d = nc.dram_tensor("own", [OWN, D], BF16, kind="ExternalInput")
    nr1_d = nc.dram_tensor("nr1", [128, NPT, D], BF16, kind="ExternalInput")
    nr2_d = nc.dram_tensor("nr2", [128, NPT, D], BF16, kind="ExternalInput")
    mf_d = nc.dram_tensor("mf", [128, 48], F32, kind="ExternalInput")
    selb_d = nc.dram_tensor("selb", [128, 8 + NB * LPC], BF16, kind="ExternalInput")
    out_d = nc.dram_tensor("out", [1, 1], F32, kind="ExternalOutput")

    groups = [list(range(M))]

    with tile.TileContext(nc) as tc:
        with (
            tc.tile_pool(name="cst", bufs=1) as cst,
            tc.tile_pool(name="sb", bufs=2) as sb,
            tc.tile_pool(name="drp", bufs=1, space="DRAM") as drp,
        ):
            # ---- constants / inputs (7 DMAs total in the whole kernel) ----
            iota_t = cst.tile([128, 128], I16, name="iota_t")
            nc.gpsimd.iota(iota_t[:], pattern=[[1, 128]], base=0, channel_multiplier=0)
            ones_bf = cst.tile([128, 1], BF16, name="ones_bf")
            nc.vector.memset(ones_bf[:], 1.0)
            ones_f32 = cst.tile([128, 1], F32, name="ones_f32")
            nc.vector.memset(ones_f32[:], 1.0)
            epsb = cst.tile([128, 1], F32, name="epsb")
            nc.vector.memset(epsb[:], 1e-12)

            mf = cst.tile([128, 48], F32, name="mf")   # krel | neg meta | lab
            nc.sync.dma_start(mf[:], mf_d[:, :])
            g2 = cst.tile([128, NPT, D], BF16, name="g2")
            nc.gpsimd.dma_start(g2[:], nr2_d[:, :, :])
            selb = cst.tile([128, 8 + NB * LPC], BF16, name="selb")
            nc.gpsimd.dma_start(selb[:], selb_d[:, :])
            # ---- phase A: row sumsq + one-hots + inv-norms ----
            # own-row DMAs issued FIRST: they gate the scalar-norm chain,
            # while the (larger) neg-pair rows are not needed until later
            o_sum = cst.tile([128, NB * 128], BF16, name="o_sum")
            nc.vector.memset(o_sum[OSL:128, :], 0.0)
            sqall = cst.tile([128, NB], F32, name="sqall")
            nc.vector.memset(sqall[OSL:128, :], 0.0)
            e_ts = []
            dma_engs = [nc.sync, nc.gpsimd]
            for j in range(NB):
                e_t = cst.tile([OSL, D], BF16, name=f"e_{j}")
                dma_engs[j % 2].dma_start(e_t[:], own_d[j * OSL : (j + 1) * OSL, :])
                e_ts.append(e_t)
            # g1 as 5 flat tiles (feeds scalar-side sumsq); g2 stays 3D
            g1t = []
            for t in range(NPT):
                g = cst.tile([128, D], BF16, name=f"g1_{t}")
                nc.scalar.dma_start(g[:], nr1_d[:, t, :])
                g1t.append(g)

            for j in range(NB):
                scr = sb.tile([OSL, D], F32, name=f"scr_{j}", tag="scr", bufs=4)
                nc.scalar.activation(
                    scr[:], e_ts[j][:], ACTF.Square,
                    accum_out=sqall[0:OSL, j : j + 1],
                )
                nc.vector.tensor_scalar(
                    o_sum[0:OSL, j * 128 : (j + 1) * 128],
                    iota_t[0:OSL, :],
                    mf[0:OSL, j : j + 1],
                    None,
                    ALU.is_equal,
                )
            # inv = 1/sqrt(max(ss, eps)); pad rows (ss=0) stay finite, one-hot=0
            sqc = cst.tile([128, NB], F32, name="sqc")
            nc.vector.tensor_scalar(sqc[:], sqall[:], 1e-12, None, ALU.max)
            nrm = cst.tile([128, NB], F32, name="nrm")
            nc.scalar.activation(nrm[:], sqc[:], ACTF.Sqrt)
            inv = cst.tile([128, NB], F32, name="inv")
            nc.vector.reciprocal(inv[:], nrm[:])

            # ---- phase B: W one-hot matmuls (scaled one-hot x raw rows) ----
            pspW_cm = tc.tile_pool(name="pspW", bufs=1, space="PSUM")
            pspW = pspW_cm.__enter__()
            pw_all = pspW.tile([128, NB * D], F32, name="pw_all")
            for j in range(NB):
                soh = sb.tile([OSL, 128], BF16, name=f"soh_{j}", tag="soh", bufs=4)
                nc.vector.tensor_scalar(
                    soh[:], o_sum[0:OSL, j * 128 : (j + 1) * 128],
                    inv[0:OSL, j : j + 1], None, ALU.mult,
                )
                nc.tensor.matmul(
                    pw_all[:, j * D : (j + 1) * D], soh[:], e_ts[j][:],
                    start=True, stop=True,
                )

            # ---- phase C: evacuate W (bf16 copies) + one big ||W||^2 ----
            wsqc = sb.tile([128, 1], F32, name="wsqc")
            wscr = sb.tile([128, NB * D], F32, name="wscr")
            nc.scalar.activation(
                wscr[:], pw_all[:, :], ACTF.Square, accum_out=wsqc[:]
            )
            w_sb = []
            for b in range(NB):
                w_t = sb.tile([128, D], BF16, name=f"w_{b}", tag=f"wsb{b}")
                if b % 2 == 0:
                    nc.vector.tensor_copy(w_t[:], pw_all[:, b * D : (b + 1) * D])
                else:
                    nc.scalar.activation(
                        w_t[:], pw_all[:, b * D : (b + 1) * D], ACTF.Copy
                    )
                w_sb.append(w_t)
            pspW_cm.__exit__(None, None, None)

            # ---- phase D: counts + U from evacuated W ----
            psp2_cm = tc.tile_pool(name="psp2", bufs=1, space="PSUM")
            psp2 = psp2_cm.__enter__()
            # partials gathered as columns of redcol, one ones-matmul at the end
            # cols: 0=||W||^2 1=neg_sum 2=neg_cnt 3=sum nk^2 4=||U||^2 5=sum nL^2
            redcol = sb.tile([128, 8], F32, name="redcol")
            nc.vector.memset(redcol[:], 0.0)

            # n_k: column sums of the one-hot -> [1, 1024] (two bank-aligned
            # matmuls, as in the proven baseline construct)
            psc = psp2.tile([1, 1024], F32, name="psc")
            nc.tensor.matmul(
                psc[0:1, 0:512], ones_bf[:], o_sum[:, 0:512], start=True, stop=True
            )
            nc.tensor.matmul(
                psc[0:1, 512:1024], ones_bf[:], o_sum[:, 512:1024],
                start=True, stop=True,
            )
            cscr = sb.tile([1, 1024], F32, name="cscr")
            nk2t = sb.tile([1, 1], F32, name="nk2t")
            nc.scalar.activation(cscr[:], psc[0:1, :], ACTF.Square, accum_out=nk2t[:])
            # n_L via label one-hots (mf cols 40:48 hold each slot's label 0-63)
            psnl = psp2.tile([1, LPC], F32, name="psnl")
            for b in range(NB):
                olab = sb.tile([OSL, LPC], BF16, name=f"olab_{b}", tag="olab", bufs=2)
                nc.vector.tensor_scalar(
                    olab[:], iota_t[0:OSL, 0:LPC], mf[0:OSL, 40 + b : 41 + b],
                    None, ALU.is_equal,
                )
                nc.tensor.matmul(
                    psnl[:, :], ones_bf[0:OSL, :], olab[:],
                    start=(b == 0), stop=(b == NB - 1),
                )
            lscr = sb.tile([1, LPC], F32, name="lscr")
            nl2t = sb.tile([1, 1], F32, name="nl2t")
            nc.scalar.activation(lscr[:], psnl[0:1, :], ACTF.Square, accum_out=nl2t[:])
            # U[64, 512] accumulated over blocks via label-selection matrix
            psU = psp2.tile([LPC, D], F32, name="psU")
            for b in range(NB):
                nc.tensor.matmul(
                    psU[:, :],
                    selb[:, 8 + b * LPC : 8 + (b + 1) * LPC],
                    w_sb[b][:],
                    start=(b == 0),
                    stop=(b == NB - 1),
                )
            uscr = sb.tile([LPC, D], F32, name="uscr")
            u2t = sb.tile([LPC, 1], F32, name="u2t")
            nc.scalar.activation(uscr[:], psU[:, :], ACTF.Square, accum_out=u2t[:])

            # ---- phase E: negative pairs (proven mult + reduce on 3D tiles) ----
            ss1 = sb.tile([128, NPT], F32, name="ss1")
            ss2 = sb.tile([128, NPT], F32, name="ss2")
            dots = sb.tile([128, NPT], F32, name="dots")
            for t in range(NPT):
                nsc = sb.tile([128, D], F32, name=f"nsc_{t}", tag="nsc", bufs=2)
                nc.scalar.activation(
                    nsc[:], g1t[t][:], ACTF.Square, accum_out=ss1[:, t : t + 1]
                )
            sq2 = sb.tile([128, NPT, D], BF16, name="sq2")
            prod = sb.tile([128, NPT, D], BF16, name="prod")
            for t in range(NPT):
                nc.vector.tensor_tensor(sq2[:, t, :], g2[:, t, :], g2[:, t, :], ALU.mult)
                nc.vector.tensor_reduce(
                    ss2[:, t : t + 1], sq2[:, t, :], axis=AX.X, op=ALU.add
                )
                nc.vector.tensor_tensor(
                    prod[:, t, :], g1t[t][:], g2[:, t, :], ALU.mult
                )
                nc.vector.tensor_reduce(
                    dots[:, t : t + 1], prod[:, t, :], axis=AX.X, op=ALU.add
                )

            ssp = sb.tile([128, NPT], F32, name="ssp")
            nc.vector.tensor_tensor(ssp[:], ss1[:], ss2[:], ALU.mult)
            nc.vector.tensor_scalar(ssp[:], ssp[:], 1e-12, None, ALU.max)
            s12 = sb.tile([128, NPT], F32, name="s12")
            nc.scalar.activation(s12[:], ssp[:], ACTF.Sqrt)
            inv12 = sb.tile([128, NPT], F32, name="inv12")
            nc.vector.reciprocal(inv12[:], s12[:])
            sim = sb.tile([128, NPT], F32, name="sim")
            nc.vector.tensor_tensor(sim[:], dots[:], inv12[:], ALU.mult)
            pen = sb.tile([128, NPT], F32, name="pen")
            nc.vector.tensor_scalar(pen[:], sim[:], 0.0, None, ALU.max)

            # masks: (l1 != l2) & (g1 != g2) & (cons1 | cons2); meta at mf[:,8:]
            mo = 8
            vmask = sb.tile([128, NPT], F32, name="vmask")
            nc.vector.tensor_tensor(
                vmask[:], mf[:, mo : mo + NPT], mf[:, mo + NPT : mo + 2 * NPT],
                ALU.not_equal,
            )
            gmask = sb.tile([128, NPT], F32, name="gmask")
            nc.vector.tensor_tensor(
                gmask[:], mf[:, mo + 2 * NPT : mo + 3 * NPT],
                mf[:, mo + 3 * NPT : mo + 4 * NPT], ALU.not_equal,
            )
            nc.vector.tensor_tensor(vmask[:], vmask[:], gmask[:], ALU.mult)
            c1c = sb.tile([128, NPT], F32, name="c1c")
            nc.vector.tensor_scalar(
                c1c[:], mf[:, mo + 4 * NPT : mo + 5 * NPT], 2.5, None, ALU.is_lt
            )
            c2c = sb.tile([128, NPT], F32, name="c2c")
            nc.vector.tensor_scalar(
                c2c[:], mf[:, mo + 5 * NPT : mo + 6 * NPT], 2.5, None, ALU.is_lt
            )
            nc.vector.tensor_tensor(c1c[:], c1c[:], c2c[:], ALU.add)
            cmask = sb.tile([128, NPT], F32, name="cmask")
            nc.vector.tensor_scalar(cmask[:], c1c[:], 0.5, None, ALU.is_gt)
            nc.vector.tensor_tensor(vmask[:], vmask[:], cmask[:], ALU.mult)
            nc.vector.tensor_tensor(pen[:], pen[:], vmask[:], ALU.mult)

            # ---- phase F: gather partials into one row via a ones-matmul ----
            nc.vector.tensor_copy(redcol[:, 0:1], wsqc[:])
            nc.vector.tensor_reduce(redcol[:, 1:2], pen[:], axis=AX.X, op=ALU.add)
            nc.vector.tensor_reduce(redcol[:, 2:3], vmask[:], axis=AX.X, op=ALU.add)
            nc.vector.tensor_copy(redcol[0:1, 3:4], nk2t[:])
            nc.vector.tensor_copy(redcol[0:LPC, 4:5], u2t[:])
            nc.vector.tensor_copy(redcol[0:1, 5:6], nl2t[:])
            psum_s = psp2.tile([1, 8], F32, name="psum_s")
            nc.tensor.matmul(
                psum_s[0:1, 0:8], ones_f32[:], redcol[:], start=True, stop=True
            )
            # ---- phase G: 2 KB all-reduce of the six scalars ----
            arb = drp.tile([8, 512], F32, name="arb")
            arbz = sb.tile([8, 512], F32, name="arbz")
            nc.vector.memset(arbz[:], 0.0)
            nc.vector.tensor_copy(arbz[0:1, 0:8], psum_s[0:1, 0:8])
            nc.sync.dma_start(arb[:, :], arbz[:])
            aro = drp.tile([8, 512], F32, name="aro", addr_space="Shared")
            nc.gpsimd.collective_compute(
                "AllReduce",
                ALU.add,
                replica_groups=groups,
                ins=[arb.opt()],
                outs=[aro.opt()],
            )

            # ---- phase H: final scalar ----
            scf = sb.tile([1, 512], F32, name="scf")
            nc.sync.dma_start(scf[:], aro[0:1, :])
            # pos_cnt = 0.5*(sum nL^2 - sum nk^2)
            pc = sb.tile([1, 1], F32, name="pc")
            nc.vector.tensor_tensor(pc[:], scf[:, 5:6], scf[:, 3:4], ALU.subtract)
            nc.vector.tensor_scalar(pc[:], pc[:], 0.5, None, ALU.mult)
            # pos_sumsim = 0.5*(||U||^2 - ||W||^2)
            ps_ = sb.tile([1, 1], F32, name="ps_")
            nc.vector.tensor_tensor(ps_[:], scf[:, 4:5], scf[:, 0:1], ALU.subtract)
            nc.vector.tensor_scalar(ps_[:], ps_[:], 0.5, None, ALU.mult)
            # pos_loss = (pos_cnt - pos_sumsim) / max(pos_cnt,1) * (pos_cnt>0)
            psum_t = sb.tile([1, 1], F32, name="psum_t")
            nc.vector.tensor_tensor(psum_t[:], pc[:], ps_[:], ALU.subtract)
            den = sb.tile([1, 1], F32, name="den")
            nc.vector.tensor_scalar(den[:], pc[:], 1.0, None, ALU.max)
            rec = sb.tile([1, 1], F32, name="rec")
            nc.vector.reciprocal(rec[:], den[:])
            msk = sb.tile([1, 1], F32, name="msk")
            nc.vector.tensor_scalar(msk[:], pc[:], 0.0, None, ALU.is_gt)
            ploss = sb.tile([1, 1], F32, name="ploss")
            nc.vector.scalar_tensor_tensor(
                ploss[:], psum_t[:], rec[:], msk[:], ALU.mult, ALU.mult
            )
            # neg_loss
            den2 = sb.tile([1, 1], F32, name="den2")
            nc.vector.tensor_scalar(den2[:], scf[:, 2:3], 1.0, None, ALU.max)
            rec2 = sb.tile([1, 1], F32, name="rec2")
            nc.vector.reciprocal(rec2[:], den2[:])
            msk2 = sb.tile([1, 1], F32, name="msk2")
            nc.vector.tensor_scalar(msk2[:], scf[:, 2:3], 0.0, None, ALU.is_gt)
            nloss = sb.tile([1, 1], F32, name="nloss")
            nc.vector.scalar_tensor_tensor(
                nloss[:], scf[:, 1:2], rec2[:], msk2[:], ALU.mult, ALU.mult
            )

            outv = sb.tile([1, 1], F32, name="outv")
            nc.vector.tensor_tensor(outv[:], ploss[:], nloss[:], ALU.add)
            nc.sync.dma_start(out_d[:, :], outv[:])
            psp2_cm.__exit__(None, None, None)

    nc.compile()
    _PROGRAM_CACHE["nc"] = nc
    return nc


def make_in_maps(embeddings, labels, graph_ids, categories, idx1, idx2):
    """Host-side sharding / layout marshaling. Returns per-core input dicts."""
    import ml_dtypes

    emb = np.ascontiguousarray(
        np.asarray(embeddings, dtype=np.float32).astype(ml_dtypes.bfloat16)
    )
    l = np.asarray(labels).astype(np.int64)
    g = np.asarray(graph_ids).astype(np.int64)
    c = np.asarray(categories).astype(np.int64)
    i1 = np.asarray(idx1).astype(np.int64)
    i2 = np.asarray(idx2).astype(np.int64)
    assert emb.shape == (N, D) and l.shape == (N,) and i1.shape == (S,)

    cons = c < 3
    p_ar = np.arange(128)
    selb = np.zeros((128, 8 + NB * LPC), dtype=ml_dtypes.bfloat16)
    selb[:, 0:8] = p_ar[:, None] // 16 == np.arange(8)[None, :]
    for b in range(NB):
        selb[:, 8 + b * LPC : 8 + (b + 1) * LPC] = (
            (8 * b + p_ar[:, None] // 16) == np.arange(LPC)[None, :]
        )

    in_maps = []
    for core in range(M):
        own = np.zeros((NB, OSL, D), dtype=ml_dtypes.bfloat16)
        mf = np.zeros((128, 48), dtype=np.float32)
        mf[:, 0:8] = 999.0
        mf[:, 40:48] = 999.0
        for b in range(NB):
            lo = 64 * core + 8 * b
            sel = np.nonzero(cons & (l >= lo) & (l < lo + 8))[0]
            nb_ = len(sel)
            assert nb_ <= OSL, f"key-block overflow: {nb_} rows"
            own[b, :nb_] = emb[sel]
            mf[:nb_, b] = ((l[sel] - lo) * 16 + g[sel]).astype(np.float32)
            mf[:nb_, 40 + b] = (l[sel] - 64 * core).astype(np.float32)

        # negative pairs: q-th pair of this core at [q % 128, q // 128]
        sl = slice(core * SP, (core + 1) * SP)
        p1 = np.zeros(NPT * 128, np.int64)
        p2 = np.zeros(NPT * 128, np.int64)
        p1[:SP] = i1[sl]
        p2[:SP] = i2[sl]
        nr1 = np.ascontiguousarray(emb[p1].reshape(NPT, 128, D).transpose(1, 0, 2))
        nr2 = np.ascontiguousarray(emb[p2].reshape(NPT, 128, D).transpose(1, 0, 2))
        for f, arr in enumerate((l[p1], l[p2], g[p1], g[p2], c[p1], c[p2])):
            mf[:, 8 + f * NPT : 8 + (f + 1) * NPT] = arr.reshape(NPT, 128).T
        # pad pairs (q >= SP): force-invalid via equal labels
        padmask = np.zeros(NPT * 128, bool)
        padmask[SP:] = True
        pm2 = padmask.reshape(NPT, 128).T
        mf[:, 8 : 8 + NPT][pm2] = 0.0
        mf[:, 8 + NPT : 8 + 2 * NPT][pm2] = 0.0

        in_maps.append(
            {
                "own": own.reshape(OWN, D),
                "nr1": nr1,
                "nr2": nr2,
                "mf": mf,
                "selb": selb,
            }
        )
    return in_maps


def kernel(embeddings, labels, graph_ids, categories, idx1, idx2):
    nc = build_program()
    in_maps = make_in_maps(embeddings, labels, graph_ids, categories, idx1, idx2)
    res = run_bass_kernel_spmd(nc, in_maps, list(range(M)))
    out = np.asarray(res.results[0]["out"], dtype=np.float32)
    return out.reshape(())


# revision 9
# speedup vs baseline: 1.1200x; 1.0804x over previous
"""Trainium2 Bass kernel for AlignmentContrastiveLoss (8-core SPMD, label-sharded).

Math: with conserved c_i = (cat_i < 3), key k_i = (label_i - 64*core)*16 +
graph_i (non-conserved rows dropped),

  pos_cnt        = 1/2 (sum_L n_L^2 - sum_k n_k^2)
  sum_valid_sims = 1/2 (||U||_F^2 - ||W||_F^2)
      U[L,:] = sum_{i: l_i=L, c_i} e_i   (e = row-normalized embeddings)
      W[k,:] = sum_{i: k_i=k, c_i} e_i
  pos_sum        = pos_cnt - sum_valid_sims

Sharding: conserved rows are bucketed BY LABEL on the host -- core c owns
labels [64c, 64c+64).  Positive pairs require equal labels, so every term
above is core-local: ||U||^2, ||W||^2, sum n_L^2 and sum n_k^2 all reduce
to per-core scalars, and the only collective is a 2 KB AllReduce of six
f32 scalars.  Per core the 1024 keys split into 8 blocks of 128; rows are
host-packed 128 slots per block (occupancy ~66; overflow needs +8 sigma),
so W is 8 single-tile one-hot matmuls with 1/||r|| folded into the
one-hot.  U = sel^T W via 8 more matmuls against a constant label-
selection matrix.  Negative pairs are sharded 625/core with raw rows
host-gathered (pure indexing); sims use fused multiply-reduce.
"""

import os
import sys

import numpy as np

if "/opt/trn_rl_repo" not in sys.path:
    sys.path.insert(0, "/opt/trn_rl_repo")

# persistent jax/neuron compile cache: repeat invocations skip the NEFF build
os.environ.setdefault("JAX_COMPILATION_CACHE_DIR", "/tmp/jaxcache")
os.environ.setdefault("JAX_PERSISTENT_CACHE_MIN_COMPILE_TIME_SECS", "1")
os.environ.setdefault("JAX_PERSISTENT_CACHE_MIN_ENTRY_SIZE_BYTES", "0")

import concourse.mybir as mybir  # noqa: E402
import concourse.tile as tile  # noqa: E402
from concourse import bacc  # noqa: E402
from concourse.bass_utils import run_bass_kernel_spmd  # noqa: E402

# Problem constants (hardcoded per the self-contained-kernel contract).
N, D, S = 8192, 512, 5000
M = 8                 # cores
NB = 8                # key blocks per core (128 keys each; 1024 keys/core)
OWN = NB * 96         # own-row slots per core (96 per key block)
OSL = 96              # slots per key block (max observed occupancy 89)
SP = S // M           # 625 pairs per core
NPT = 5               # neg pair tiles: 5 * 128 = 640 >= 625
LPC = 64              # labels per core

F32 = mybir.dt.float32
BF16 = mybir.dt.bfloat16
I16 = mybir.dt.int16
ALU = mybir.AluOpType
ACTF = mybir.ActivationFunctionType
AX = mybir.AxisListType

_PROGRAM_CACHE = {}


def build_program():
    """Build + compile the (single) SPMD Bass program. Returns nc."""
    if "nc" in _PROGRAM_CACHE:
        return _PROGRAM_CACHE["nc"]

    nc = bacc.Bacc("TRN2", target_bir_lowering=False, debug=False, num_devices=M)

    own_d = nc.dram_tensor("own", [OWN, D], BF16, kind="ExternalInput")
    nr1_d = nc.dram_tensor("nr1", [128, NPT, D], BF16, kind="ExternalInput")
    nr2_d = nc.dram_tensor("nr2", [128, NPT, D], BF16, kind="ExternalInput")
    mf_d = nc.dram_tensor("mf", [128, 48], F32, kind="ExternalInput")
    selb_d = nc.dram_tensor("selb", [128, 8 + NB * LPC], BF16, kind="ExternalInput")
    out_d = nc.dram_tensor("out", [1, 1], F32, kind="ExternalOutput")

    groups = [list(range(M))]

    with tile.TileContext(nc) as tc:
        with (
            tc.tile_pool(name="cst", bufs=1) as cst,
            tc.tile_pool(name="sb", bufs=2) as sb,
            tc.tile_pool(name="drp", bufs=1, space="DRAM") as drp,
        ):
            # ---- constants / inputs (7 DMAs total in the whole kernel) ----
            iota_t = cst.tile([128, 128], I16, name="iota_t")
            nc.gpsimd.iota(iota_t[:], pattern=[[1, 128]], base=0, channel_multiplier=0)
            ones_bf = cst.tile([128, 1], BF16, name="ones_bf")
            nc.vector.memset(ones_bf[:], 1.0)
            ones_f32 = cst.tile([128, 1], F32, name="ones_f32")
            nc.vector.memset(ones_f32[:], 1.0)
            epsb = cst.tile([128, 1], F32, name="epsb")
            nc.vector.memset(epsb[:], 1e-12)

            mf = cst.tile([128, 48], F32, name="mf")   # krel | neg meta | lab
            nc.sync.dma_start(mf[:], mf_d[:, :])
            g2 = cst.tile([128, NPT, D], BF16, name="g2")
            nc.gpsimd.dma_start(g2[:], nr2_d[:, :, :])
            selb = cst.tile([128, 8 + NB * LPC], BF16, name="selb")
            nc.gpsimd.dma_start(selb[:], selb_d[:, :])
            # ---- phase A: row sumsq + one-hots + inv-norms ----
            # own-row DMAs issued FIRST: they gate the scalar-norm chain,
            # while the (larger) neg-pair rows are not needed until later
            o_sum = cst.tile([128, NB * 128], BF16, name="o_sum")
            nc.vector.memset(o_sum[OSL:128, :], 0.0)
            sqall = cst.tile([128, NB], F32, name="sqall")
            nc.vector.memset(sqall[OSL:128, :], 0.0)
            e_ts = []
            dma_engs = [nc.sync, nc.gpsimd]
            for j in range(NB):
                e_t = cst.tile([OSL, D], BF16, name=f"e_{j}")
                dma_engs[j % 2].dma_start(e_t[:], own_d[j * OSL : (j + 1) * OSL, :])
                e_ts.append(e_t)
            # g1 as 5 flat tiles (feeds scalar-side sumsq); g2 stays 3D
            g1t = []
            for t in range(NPT):
                g = cst.tile([128, D], BF16, name=f"g1_{t}")
                nc.scalar.dma_start(g[:], nr1_d[:, t, :])
                g1t.append(g)

            for j in range(NB):
                scr = sb.tile([OSL, D], F32, name=f"scr_{j}", tag="scr", bufs=4)
                nc.scalar.activation(
                    scr[:], e_ts[j][:], ACTF.Square,
                    accum_out=sqall[0:OSL, j : j + 1],
                )
                nc.vector.tensor_scalar(
                    o_sum[0:OSL, j * 128 : (j + 1) * 128],
                    iota_t[0:OSL, :],
                    mf[0:OSL, j : j + 1],
                    None,
                    ALU.is_equal,
                )
            # inv = 1/sqrt(max(ss, eps)); pad rows (ss=0) stay finite, one-hot=0
            sqc = cst.tile([128, NB], F32, name="sqc")
            nc.vector.tensor_scalar(sqc[:], sqall[:], 1e-12, None, ALU.max)
            nrm = cst.tile([128, NB], F32, name="nrm")
            nc.scalar.activation(nrm[:], sqc[:], ACTF.Sqrt)
            inv = cst.tile([128, NB], F32, name="inv")
            nc.vector.reciprocal(inv[:], nrm[:])

            # ---- phase B: W one-hot matmuls (scaled one-hot x raw rows) ----
            pspW_cm = tc.tile_pool(name="pspW", bufs=1, space="PSUM")
            pspW = pspW_cm.__enter__()
            pw_all = pspW.tile([128, NB * D], F32, name="pw_all")
            for j in range(NB):
                soh = sb.tile([OSL, 128], BF16, name=f"soh_{j}", tag="soh", bufs=4)
                nc.vector.tensor_scalar(
                    soh[:], o_sum[0:OSL, j * 128 : (j + 1) * 128],
                    inv[0:OSL, j : j + 1], None, ALU.mult,
                )
                nc.tensor.matmul(
                    pw_all[:, j * D : (j + 1) * D], soh[:], e_ts[j][:],
                    start=True, stop=True,
                )

            # ---- phase C: evacuate W (bf16 copies) + one big ||W||^2 ----
            wsqc = sb.tile([128, 1], F32, name="wsqc")
            wscr = sb.tile([128, NB * D], F32, name="wscr")
            nc.scalar.activation(
                wscr[:], pw_all[:, :], ACTF.Square, accum_out=wsqc[:]
            )
            w_sb = []
            for b in range(NB):
                w_t = sb.tile([128, D], BF16, name=f"w_{b}", tag=f"wsb{b}")
                if b % 2 == 0:
                    nc.vector.tensor_copy(w_t[:], pw_all[:, b * D : (b + 1) * D])
                else:
                    nc.scalar.activation(
                        w_t[:], pw_all[:, b * D : (b + 1) * D], ACTF.Copy
                    )
                w_sb.append(w_t)
            pspW_cm.__exit__(None, None, None)

            # ---- phase D: counts + U from evacuated W ----
            psp2_cm = tc.tile_pool(name="psp2", bufs=1, space="PSUM")
            psp2 = psp2_cm.__enter__()
            # partials gathered as columns of redcol, one ones-matmul at the end
            # cols: 0=||W||^2 1=neg_sum 2=neg_cnt 3=sum nk^2 4=||U||^2 5=sum nL^2
            redcol = sb.tile([128, 8], F32, name="redcol")
            nc.vector.memset(redcol[:], 0.0)

            # n_k: column sums of the one-hot -> [1, 1024] (two bank-aligned
            # matmuls, as in the proven baseline construct)
            psc = psp2.tile([1, 1024], F32, name="psc")
            nc.tensor.matmul(
                psc[0:1, 0:512], ones_bf[:], o_sum[:, 0:512], start=True, stop=True
            )
            nc.tensor.matmul(
                psc[0:1, 512:1024], ones_bf[:], o_sum[:, 512:1024],
                start=True, stop=True,
            )
            cscr = sb.tile([1, 1024], F32, name="cscr")
            nk2t = sb.tile([1, 1], F32, name="nk2t")
            nc.scalar.activation(cscr[:], psc[0:1, :], ACTF.Square, accum_out=nk2t[:])
            # n_L via label one-hots (mf cols 40:48 hold each slot's label 0-63)
            psnl = psp2.tile([1, LPC], F32, name="psnl")
            for b in range(NB):
                olab = sb.tile([OSL, LPC], BF16, name=f"olab_{b}", tag="olab", bufs=2)
                nc.vector.tensor_scalar(
                    olab[:], iota_t[0:OSL, 0:LPC], mf[0:OSL, 40 + b : 41 + b],
                    None, ALU.is_equal,
                )
                nc.tensor.matmul(
                    psnl[:, :], ones_bf[0:OSL, :], olab[:],
                    start=(b == 0), stop=(b == NB - 1),
                )
            lscr = sb.tile([1, LPC], F32, name="lscr")
            nl2t = sb.tile([1, 1], F32, name="nl2t")
            nc.scalar.activation(lscr[:], psnl[0:1, :], ACTF.Square, accum_out=nl2t[:])
            # U[64, 512] accumulated over blocks via label-selection matrix
            psU = psp2.tile([LPC, D], F32, name="psU")
            for b in range(NB):
                nc.tensor.matmul(
                    psU[:, :],
                    selb[:, 8 + b * LPC : 8 + (b + 1) * LPC],
                    w_sb[b][:],
                    start=(b == 0),
                    stop=(b == NB - 1),
                )
            uscr = sb.tile([LPC, D], F32, name="uscr")
            u2t = sb.tile([LPC, 1], F32, name="u2t")
            nc.scalar.activation(uscr[:], psU[:, :], ACTF.Square, accum_out=u2t[:])

            # ---- phase E: negative pairs (proven mult + reduce on 3D tiles) ----
            ss1 = sb.tile([128, NPT], F32, name="ss1")
            ss2 = sb.tile([128, NPT], F32, name="ss2")
            dots = sb.tile([128, NPT], F32, name="dots")
            for t in range(NPT):
                nsc = sb.tile([128, D], F32, name=f"nsc_{t}", tag="nsc", bufs=2)
                nc.scalar.activation(
                    nsc[:], g1t[t][:], ACTF.Square, accum_out=ss1[:, t : t + 1]
                )
            sq2 = sb.tile([128, NPT, D], BF16, name="sq2")
            prod = sb.tile([128, NPT, D], BF16, name="prod")
            for t in range(NPT):
                nc.vector.tensor_tensor(sq2[:, t, :], g2[:, t, :], g2[:, t, :], ALU.mult)
                nc.vector.tensor_reduce(
                    ss2[:, t : t + 1], sq2[:, t, :], axis=AX.X, op=ALU.add
                )
                nc.vector.tensor_tensor(
                    prod[:, t, :], g1t[t][:], g2[:, t, :], ALU.mult
                )
                nc.vector.tensor_reduce(
                    dots[:, t : t + 1], prod[:, t, :], axis=AX.X, op=ALU.add
                )

            ssp = sb.tile([128, NPT], F32, name="ssp")
            nc.vector.tensor_tensor(ssp[:], ss1[:], ss2[:], ALU.mult)
            nc.vector.tensor_scalar(ssp[:], ssp[:], 1e-12, None, ALU.max)
            s12 = sb.tile([128, NPT], F32, name="s12")
            nc.scalar.activation(s12[:], ssp[:], ACTF.Sqrt)
            inv12 = sb.tile([128, NPT], F32, name="inv12")
            nc.vector.reciprocal(inv12[:], s12[:])
            sim = sb.tile([128, NPT], F32, name="sim")
            nc.vector.tensor_tensor(sim[:], dots[:], inv12[:], ALU.mult)
            pen = sb.tile([128, NPT], F32, name="pen")
            nc.vector.tensor_scalar(pen[:], sim[:], 0.0, None, ALU.max)

            # masks: (l1 != l2) & (g1 != g2) & (cons1 | cons2); meta at mf[:,8:]
            mo = 8
            vmask = sb.tile([128, NPT], F32, name="vmask")
            nc.vector.tensor_tensor(
                vmask[:], mf[:, mo : mo + NPT], mf[:, mo + NPT : mo + 2 * NPT],
                ALU.not_equal,
            )
            gmask = sb.tile([128, NPT], F32, name="gmask")
            nc.vector.tensor_tensor(
                gmask[:], mf[:, mo + 2 * NPT : mo + 3 * NPT],
                mf[:, mo + 3 * NPT : mo + 4 * NPT], ALU.not_equal,
            )
            nc.vector.tensor_tensor(vmask[:], vmask[:], gmask[:], ALU.mult)
            c1c = sb.tile([128, NPT], F32, name="c1c")
            nc.vector.tensor_scalar(
                c1c[:], mf[:, mo + 4 * NPT : mo + 5 * NPT], 2.5, None, ALU.is_lt
            )
            c2c = sb.tile([128, NPT], F32, name="c2c")
            nc.vector.tensor_scalar(
                c2c[:], mf[:, mo + 5 * NPT : mo + 6 * NPT], 2.5, None, ALU.is_lt
            )
            nc.vector.tensor_tensor(c1c[:], c1c[:], c2c[:], ALU.add)
            cmask = sb.tile([128, NPT], F32, name="cmask")
            nc.vector.tensor_scalar(cmask[:], c1c[:], 0.5, None, ALU.is_gt)
            nc.vector.tensor_tensor(vmask[:], vmask[:], cmask[:], ALU.mult)
            nc.vector.tensor_tensor(pen[:], pen[:], vmask[:], ALU.mult)

            # ---- phase F: gather partials into one row via a ones-matmul ----
            nc.vector.tensor_copy(redcol[0:1, 0:1], nl2t[:])
            nc.vector.tensor_copy(redcol[0:LPC, 1:2], u2t[:])
            nc.vector.tensor_copy(redcol[0:1, 2:3], nk2t[:])
            nc.vector.tensor_copy(redcol[:, 3:4], wsqc[:])
            nc.vector.tensor_reduce(redcol[:, 4:5], pen[:], axis=AX.X, op=ALU.add)
            nc.vector.tensor_reduce(redcol[:, 5:6], vmask[:], axis=AX.X, op=ALU.add)
            psum_s = psp2.tile([1, 8], F32, name="psum_s")
            nc.tensor.matmul(
                psum_s[0:1, 0:8], ones_f32[:], redcol[:], start=True, stop=True
            )
            # ---- phase G: 2 KB all-reduce of the six scalars ----
            arb = drp.tile([8, 512], F32, name="arb")
            arbz = sb.tile([8, 512], F32, name="arbz")
            nc.vector.memset(arbz[:], 0.0)
            nc.vector.tensor_copy(arbz[0:1, 0:8], psum_s[0:1, 0:8])
            nc.sync.dma_start(arb[:, :], arbz[:])
            aro = drp.tile([8, 512], F32, name="aro", addr_space="Shared")
            nc.gpsimd.collective_compute(
                "AllReduce",
                ALU.add,
                replica_groups=groups,
                ins=[arb.opt()],
                outs=[aro.opt()],
            )

            # ---- phase H: final scalar ----
            scf = sb.tile([1, 512], F32, name="scf")
            nc.sync.dma_start(scf[:], aro[0:1, :])
            # scf cols: 0=nl2 1=U2 2=nk2 3=W2 4=neg_sum 5=neg_cnt
            # d2 = [nl2-nk2, U2-W2]; dh = d2/2 = [pos_cnt, pos_sumsim]
            dh = sb.tile([1, 2], F32, name="dh")
            nc.vector.tensor_tensor(dh[:], scf[:, 0:2], scf[:, 2:4], ALU.subtract)
            nc.vector.tensor_scalar(dh[:], dh[:], 0.5, None, ALU.mult)
            # nums = [pos_cnt - pos_sumsim, neg_sum]; dens = [pos_cnt, neg_cnt]
            nums = sb.tile([1, 2], F32, name="nums")
            nc.vector.tensor_tensor(nums[:, 0:1], dh[:, 0:1], dh[:, 1:2], ALU.subtract)
            nc.vector.tensor_copy(nums[:, 1:2], scf[:, 4:5])
            dens = sb.tile([1, 2], F32, name="dens")
            nc.vector.tensor_copy(dens[:, 0:1], dh[:, 0:1])
            nc.vector.tensor_copy(dens[:, 1:2], scf[:, 5:6])
            dmx = sb.tile([1, 2], F32, name="dmx")
            nc.vector.tensor_scalar(dmx[:], dens[:], 1.0, None, ALU.max)
            rc = sb.tile([1, 2], F32, name="rc")
            nc.vector.reciprocal(rc[:], dmx[:])
            mk = sb.tile([1, 2], F32, name="mk")
            nc.vector.tensor_scalar(mk[:], dens[:], 0.0, None, ALU.is_gt)
            ls = sb.tile([1, 2], F32, name="ls")
            nc.vector.tensor_tensor(ls[:], nums[:], rc[:], ALU.mult)
            nc.vector.tensor_tensor(ls[:], ls[:], mk[:], ALU.mult)
            outv = sb.tile([1, 1], F32, name="outv")
            nc.vector.tensor_tensor(outv[:], ls[:, 0:1], ls[:, 1:2], ALU.add)
            nc.sync.dma_start(out_d[:, :], outv[:])
            psp2_cm.__exit__(None, None, None)

    nc.compile()
    _PROGRAM_CACHE["nc"] = nc
    return nc


def make_in_maps(embeddings, labels, graph_ids, categories, idx1, idx2):
    """Host-side sharding / layout marshaling. Returns per-core input dicts."""
    import ml_dtypes

    emb = np.ascontiguousarray(
        np.asarray(embeddings, dtype=np.float32).astype(ml_dtypes.bfloat16)
    )
    l = np.asarray(labels).astype(np.int64)
    g = np.asarray(graph_ids).astype(np.int64)
    c = np.asarray(categories).astype(np.int64)
    i1 = np.asarray(idx1).astype(np.int64)
    i2 = np.asarray(idx2).astype(np.int64)
    assert emb.shape == (N, D) and l.shape == (N,) and i1.shape == (S,)

    cons = c < 3
    p_ar = np.arange(128)
    selb = np.zeros((128, 8 + NB * LPC), dtype=ml_dtypes.bfloat16)
    selb[:, 0:8] = p_ar[:, None] // 16 == np.arange(8)[None, :]
    for b in range(NB):
        selb[:, 8 + b * LPC : 8 + (b + 1) * LPC] = (
            (8 * b + p_ar[:, None] // 16) == np.arange(LPC)[None, :]
        )

    in_maps = []
    for core in range(M):
        own = np.zeros((NB, OSL, D), dtype=ml_dtypes.bfloat16)
        mf = np.zeros((128, 48), dtype=np.float32)
        mf[:, 0:8] = 999.0
        mf[:, 40:48] = 999.0
        for b in range(NB):
            lo = 64 * core + 8 * b
            sel = np.nonzero(cons & (l >= lo) & (l < lo + 8))[0]
            nb_ = len(sel)
            assert nb_ <= OSL, f"key-block overflow: {nb_} rows"
            own[b, :nb_] = emb[sel]
            mf[:nb_, b] = ((l[sel] - lo) * 16 + g[sel]).astype(np.float32)
            mf[:nb_, 40 + b] = (l[sel] - 64 * core).astype(np.float32)

        # negative pairs: q-th pair of this core at [q % 128, q // 128]
        sl = slice(core * SP, (core + 1) * SP)
        p1 = np.zeros(NPT * 128, np.int64)
        p2 = np.zeros(NPT * 128, np.int64)
        p1[:SP] = i1[sl]
        p2[:SP] = i2[sl]
        nr1 = np.ascontiguousarray(emb[p1].reshape(NPT, 128, D).transpose(1, 0, 2))
        nr2 = np.ascontiguousarray(emb[p2].reshape(NPT, 128, D).transpose(1, 0, 2))
        for f, arr in enumerate((l[p1], l[p2], g[p1], g[p2], c[p1], c[p2])):
            mf[:, 8 + f * NPT : 8 + (f + 1) * NPT] = arr.reshape(NPT, 128).T
        # pad pairs (q >= SP): force-invalid via equal labels
        padmask = np.zeros(NPT * 128, bool)
        padmask[SP:] = True
        pm2 = padmask.reshape(NPT, 128).T
        mf[:, 8 : 8 + NPT][pm2] = 0.0
        mf[:, 8 + NPT : 8 + 2 * NPT][pm2] = 0.0

        in_maps.append(
            {
                "own": own.reshape(OWN, D),
                "nr1": nr1,
                "nr2": nr2,
                "mf": mf,
                "selb": selb,
            }
        )
    return in_maps


def kernel(embeddings, labels, graph_ids, categories, idx1, idx2):
    nc = build_program()
    in_maps = make_in_maps(embeddings, labels, graph_ids, categories, idx1, idx2)
    res = run_bass_kernel_spmd(nc, in_maps, list(range(M)))
    out = np.asarray(res.results[0]["out"], dtype=np.float32)
    return out.reshape(())
